# revision 24
# baseline (speedup 1.0000x reference)
"""Trainium2 Bass kernel for nn_CentralAttentiveModule.

Math (see reference):
    v = x@Wv.T+bv ; k = x@Wk.T(+bk, cancels in softmax) ; q = x@Wq.T(+bq)
    qseg = segment_max(q) ; M = sum(qseg[cluster]*k, -1)
    attn = segment_softmax(M) ; h = attn[:,None]*v
    out = relu(batchnorm(h))

Layout: points sorted by cluster on host; clusters size-sorted and dealt
round-robin to 16 strips (8 cores x 2 partition halves; feature-major:
partition = feature x strip, free = slot).  Each cluster's points are
padded to one fixed window of ceil(cnt/GRID)*GRID slots, so every
segment max / sum / broadcast is a chunk-local fixed-window vector op.
All strips share one region schedule (per window-size counts maxed over
strips).

Per chunk: qp = Wq x (PE), qs = window-max(qp) (DVE), u = Wk^T(qs+bq)
(PE, bias folded in as rank-1), xu = u[win]*x (Pool), M = colsum(xu)
(PE block-ones mm), e = exp(M) (Act), den = window-sum(e) - npad (DVE),
at = e/den (Pool), ht = (vp+bv)*at (DVE, fused sum), sq accum (Act).
Pad slots are NOT masked: they contribute exp(0)=1 to den (subtracted
via the precomputed npad table) and bv/den to the BN sums (subtracted
via on-device npad*iv reductions + the host fake-window correction).
BN stats AllReduced across the 8 cores in-kernel; pass 4 applies
relu(A*h+B) split across DVE (f16 4x fast mode) and Act.
"""
import numpy as np
import ml_dtypes

import concourse.bacc as bacc
import concourse.tile as tile
from concourse import mybir
from concourse.bass_utils import run_bass_kernel_spmd

N_TOT = 500_000
D = 64
C_TOT = 10_000
NCORES = 8
NSTRIPS = 16
GRID = 4
LMAX = 512
BN_EPS = 1e-5
F32 = mybir.dt.float32
F16 = mybir.dt.float16
BF16 = mybir.dt.bfloat16
BF = ml_dtypes.bfloat16
FH = np.float16


# ----------------------------------------------------------------- host prep
def _host_prep(cluster):
    counts = np.bincount(cluster, minlength=C_TOT)
    order = np.argsort(cluster, kind="stable")
    pt_start = np.concatenate([[0], np.cumsum(counts)])
    wb = np.maximum((counts + GRID - 1) // GRID, 1) * GRID
    assert int(wb.max()) <= LMAX

    rank = np.argsort(-wb, kind="stable")
    strips = [rank[s::NSTRIPS] for s in range(NSTRIPS)]

    vals = sorted(set(wb.tolist()), reverse=True)
    prof = {v: max(int((wb[st] == v).sum()) for st in strips) for v in vals}

    # region schedule shared by every strip/core: (slot_off, L, v, nwin)
    schedule = []
    off = 0
    for v in vals:
        total = prof[v] * v
        lmax = (LMAX // v) * v
        o = 0
        while o < total:
            L = min(lmax, total - o)
            schedule.append((off + o, L, v, L // v))
            o += L
        off += total
    W = off
    nwtot = sum(c[3] for c in schedule)
    return dict(counts=counts, order=order, pt_start=pt_start, wb=wb,
                strips=strips, vals=vals, prof=prof, schedule=schedule, W=W,
                nwtot=nwtot)


def _strip_layout(prep, s):
    """slots/pts mapping + per-window pad counts + fake count for strip s."""
    counts, order, pt_start = prep["counts"], prep["order"], prep["pt_start"]
    wb, vals, prof, W = prep["wb"], prep["vals"], prep["prof"], prep["W"]
    cl = prep["strips"][s]

    slot_list, pt_list, npads = [], [], []
    nfake = 0
    off = 0
    for v in vals:
        mine = cl[wb[cl] == v]
        for w in range(prof[v]):
            ws = off + w * v
            if w < len(mine) and counts[mine[w]] > 0:
                c = mine[w]
                cnt = int(counts[c])
                slot_list.append(np.arange(ws, ws + cnt))
                pt_list.append(order[pt_start[c]: pt_start[c] + cnt])
                npads.append(v - cnt)
            else:
                npads.append(v - 1)  # fake window: den = v - (v-1) = 1
                nfake += 1
        off += prof[v] * v
    slots = (np.concatenate(slot_list) if slot_list else np.zeros(0, np.int64))
    pts = (np.concatenate(pt_list) if pt_list else np.zeros(0, np.int64))
    return slots, pts, np.asarray(npads, np.float32), nfake


# ------------------------------------------------------------- build program
def _build_program(W, schedule, nwtot):
    nchunks = len(schedule)
    nwoff = np.concatenate([[0], np.cumsum([c[3] for c in schedule])])
    nc = bacc.Bacc("TRN2", target_bir_lowering=False, debug=False,
                   num_devices=NCORES)

    def din(name, shape, dt=F32):
        return nc.dram_tensor(name, shape, dt, kind="ExternalInput")

    xin = din("xin", [128, W], BF16)
    wqt = din("wqt", [128, 64], BF16)
    wk2 = din("wk2", [128, 64], BF16)
    wvt = din("wvt", [128, 64], BF16)
    e2big = din("e2big", [128, 128], BF16)
    npadw = din("npadw", [128, nwtot], F16)
    bq2 = din("bq2", [128, 1])
    bv2 = din("bv2", [128, 1])
    gamma2 = din("gamma2", [128, 1])
    beta2 = din("beta2", [128, 1])
    fakecorr = din("fakecorr", [128, 2])
    hout = nc.dram_tensor("hout", [128, W], F16, kind="ExternalOutput")

    MM = dict(skip_group_check=True)

    with tile.TileContext(nc, pool_alloc_mode="queue") as tc:
        with tc.tile_pool(name="const", bufs=1) as cpool, \
             tc.tile_pool(name="p2x", bufs=6) as p2x, \
             tc.tile_pool(name="scr", bufs=6) as scr, \
             tc.tile_pool(name="small", bufs=6) as small, \
             tc.tile_pool(name="htp", bufs=nchunks + 1) as htp, \
             tc.tile_pool(name="sums", bufs=1) as sums, \
             tc.tile_pool(name="psq", bufs=3, space="PSUM") as psq, \
             tc.tile_pool(name="psv", bufs=4, space="PSUM") as psv, \
             tc.tile_pool(name="psu", bufs=1, space="PSUM") as psu, \
             tc.tile_pool(name="dram", bufs=2, space="DRAM") as dram:
            c_wqt = cpool.tile([128, 64], BF16)
            nc.scalar.dma_start(c_wqt[:], wqt[:])
            c_wk2 = cpool.tile([128, 64], BF16)
            nc.scalar.dma_start(c_wk2[:], wk2[:])
            c_wvt = cpool.tile([128, 64], BF16)
            nc.scalar.dma_start(c_wvt[:], wvt[:])
            c_e2big = cpool.tile([128, 128], BF16)
            nc.gpsimd.dma_start(c_e2big[:], e2big[:])
            c_np = cpool.tile([128, nwtot], F16)
            half = (nwtot // 2) & ~1
            nc.sync.dma_start(c_np[:, :half], npadw[:, :half])
            nc.scalar.dma_start(c_np[:, half:], npadw[:, half:])
            c_bq2 = cpool.tile([128, 1], F32)
            nc.gpsimd.dma_start(c_bq2[:], bq2[:])
            c_bv2 = cpool.tile([128, 1], F32)
            nc.scalar.dma_start(c_bv2[:], bv2[:])

            # preload Act function tables (Sqrt/Relu) so the BN epilogue
            # doesn't pay ACT_TABLE_LOAD on the critical path
            tl = sums.tile([128, 2], F32)
            nc.vector.memset(tl[:], 1.0)
            nc.scalar.activation(out=tl[:, 0:1], in_=tl[:, 0:1],
                                 func=mybir.ActivationFunctionType.Sqrt)
            nc.scalar.activation(out=tl[:, 1:2], in_=tl[:, 1:2],
                                 func=mybir.ActivationFunctionType.Relu)
            sumh = sums.tile([128, nchunks], F32)
            sumsq = sums.tile([128, nchunks], F32)
            ivall = sums.tile([128, nwtot], F32)

            # warmup collective: absorb ring-setup latency during streaming
            win = dram.tile([128, 2], F32, tag="win")
            wout = dram.tile([128, 2], F32, tag="wout")
            warm = sums.tile([128, 2], F32)
            nc.vector.memset(warm[:], 0.0)
            nc.gpsimd.dma_start(win[:], warm[:])
            nc.gpsimd.collective_compute(
                "AllReduce", mybir.AluOpType.add,
                replica_groups=[list(range(NCORES))],
                ins=[win.opt()], outs=[wout.opt()])

            state = [None] * nchunks  # (vp, et) skew carry

            def stage_a(j):
                off, L, v, nw = schedule[j]
                sl = slice(off, off + L)
                xt = p2x.tile([128, LMAX], BF16, tag="xt")
                nc.sync.dma_start(xt[:, :L], xin[:, sl])
                qp = psq.tile([128, LMAX], F32, space="PSUM", tag="qp")
                nc.tensor.matmul(out=qp[0:64, :L], lhsT=c_wqt[0:64, :],
                                 rhs=xt[0:64, :L], start=True, stop=False,
                                 tile_position=(0, 0), **MM)
                nc.tensor.matmul(out=qp[64:128, :L], lhsT=c_wqt[64:128, :],
                                 rhs=xt[64:128, :L], start=True, stop=True,
                                 tile_position=(64, 64), **MM)
                vp = psv.tile([128, LMAX], F32, space="PSUM", tag="vp")
                nc.tensor.matmul(out=vp[0:64, :L], lhsT=c_wvt[0:64, :],
                                 rhs=xt[0:64, :L], start=True, stop=False,
                                 tile_position=(0, 0), **MM)
                nc.tensor.matmul(out=vp[64:128, :L], lhsT=c_wvt[64:128, :],
                                 rhs=xt[64:128, :L], start=True, stop=True,
                                 tile_position=(64, 64), **MM)
                # window max of q -> pooled query
                qs = small.tile([128, 128], F32, tag="qs")
                nc.vector.tensor_reduce(
                    out=qs[:, :nw],
                    in_=qp[:, :L].rearrange("p (n l) -> p n l", l=v),
                    axis=mybir.AxisListType.X, op=mybir.AluOpType.max)
                qsb = small.tile([128, 128], BF16, tag="qsb")
                nc.scalar.activation(
                    out=qsb[:, :nw], in_=qs[:, :nw],
                    func=mybir.ActivationFunctionType.Identity,
                    bias=c_bq2[:], scale=1.0)
                # u = Wk^T (qs + bq)
                up = psu.tile([128, 128], F32, space="PSUM", tag="up")
                nc.tensor.matmul(out=up[0:64, :nw], lhsT=c_wk2[0:64, :],
                                 rhs=qsb[0:64, :nw], start=True, stop=False,
                                 tile_position=(0, 0), **MM)
                nc.tensor.matmul(out=up[64:128, :nw], lhsT=c_wk2[64:128, :],
                                 rhs=qsb[64:128, :nw], start=True, stop=True,
                                 tile_position=(64, 64), **MM)
                ub = small.tile([128, 128], BF16, tag="ub")
                nc.scalar.copy(out=ub[:, :nw], in_=up[:, :nw])
                # xu = u[window] * x  (Pool engine, SBUF-only operands)
                xu = scr.tile([128, LMAX], BF16, tag="xu")
                nc.gpsimd.tensor_tensor(
                    out=xu[:, :L].rearrange("p (n l) -> p n l", l=v),
                    in0=ub[:, :nw].to_broadcast([128, nw, v]),
                    in1=xt[:, :L].rearrange("p (n l) -> p n l", l=v),
                    op=mybir.AluOpType.mult)
                mp = qp  # reuse qp's PSUM bank (qp dead after the qs reduce)
                nc.tensor.matmul(out=mp[:, :L], lhsT=c_e2big[:], rhs=xu[:, :L],
                                 start=True, stop=True, **MM)
                et = scr.tile([128, LMAX], BF16, tag="et")
                nc.scalar.activation(out=et[:, :L], in_=mp[:, :L],
                                     func=mybir.ActivationFunctionType.Exp)
                state[j] = (vp, et)

            def stage_b(j):
                off, L, v, nw = schedule[j]
                wsl = slice(nwoff[j], nwoff[j] + nw)
                vp, et = state[j]
                dn = small.tile([128, 128], F32, tag="dn")
                nc.vector.tensor_reduce(
                    out=dn[:, :nw],
                    in_=et[:, :L].rearrange("p (n l) -> p n l", l=v),
                    axis=mybir.AxisListType.X, op=mybir.AluOpType.add)
                # subtract pad-slot contribution (each pad adds exp(0)=1)
                nc.vector.tensor_tensor(
                    out=dn[:, :nw], in0=dn[:, :nw], in1=c_np[:, wsl],
                    op=mybir.AluOpType.subtract)
                # 1/den straight into the strip-wide table (reused for the
                # one-shot pad-pollution correction in the epilogue)
                nc.vector.reciprocal(out=ivall[:, wsl], in_=dn[:, :nw])
                # at = e / den on the Pool engine (SBUF-only)
                at = scr.tile([128, LMAX], F32, tag="at")
                nc.gpsimd.tensor_tensor(
                    out=at[:, :L].rearrange("p (n l) -> p n l", l=v),
                    in0=ivall[:, wsl].to_broadcast([128, nw, v]),
                    in1=et[:, :L].rearrange("p (n l) -> p n l", l=v),
                    op=mybir.AluOpType.mult)
                ht = htp.tile([128, LMAX], F16, tag="ht")
                nc.vector.scalar_tensor_tensor(
                    out=ht[:, :L], in0=vp[:, :L], scalar=c_bv2[:],
                    in1=at[:, :L], op0=mybir.AluOpType.add,
                    op1=mybir.AluOpType.mult, accum_out=sumh[:, j:j + 1])
                sq = scr.tile([128, LMAX], F16, tag="sq")
                nc.scalar.activation(out=sq[:, :L], in_=ht[:, :L],
                                     func=mybir.ActivationFunctionType.Square,
                                     accum_out=sumsq[:, j:j + 1])
                state[j] = ht

            SKEW = 4
            for j in range(nchunks + SKEW):
                if j >= SKEW:
                    stage_b(j - SKEW)
                if j < nchunks:
                    stage_a(j)

            # BN stats: fold chunks, pad/fake fix, fold strips, AllReduce
            st = sums.tile([128, 2], F32)
            nc.vector.tensor_reduce(out=st[:, 0:1], in_=sumh[:],
                                    axis=mybir.AxisListType.X,
                                    op=mybir.AluOpType.add)
            nc.vector.tensor_reduce(out=st[:, 1:2], in_=sumsq[:],
                                    axis=mybir.AxisListType.X,
                                    op=mybir.AluOpType.add)
            # one-shot pad corrections: c1=sum(npad/den), c2=sum(npad/den^2)
            sA = sums.tile([128, nwtot], F32)
            nc.vector.tensor_tensor(out=sA[:], in0=c_np[:], in1=ivall[:],
                                    op=mybir.AluOpType.mult)
            cc = sums.tile([128, 2], F32)
            nc.vector.tensor_reduce(out=cc[:, 0:1], in_=sA[:],
                                    axis=mybir.AxisListType.X,
                                    op=mybir.AluOpType.add)
            nc.vector.tensor_tensor(out=sA[:], in0=sA[:], in1=ivall[:],
                                    op=mybir.AluOpType.mult)
            nc.vector.tensor_reduce(out=cc[:, 1:2], in_=sA[:],
                                    axis=mybir.AxisListType.X,
                                    op=mybir.AluOpType.add)
            # cc[:,0] *= bv ; cc[:,1] *= bv^2  (per-feature)
            bvv = sums.tile([128, 2], F32)
            nc.vector.tensor_tensor(out=bvv[:, 0:1], in0=c_bv2[:],
                                    in1=c_bv2[:], op=mybir.AluOpType.mult)
            nc.vector.tensor_scalar_mul(out=cc[:, 0:1], in0=cc[:, 0:1],
                                        scalar1=c_bv2[:])
            nc.vector.tensor_scalar_mul(out=cc[:, 1:2], in0=cc[:, 1:2],
                                        scalar1=bvv[:, 0:1])
            nc.vector.tensor_tensor(out=st[:], in0=st[:], in1=cc[:],
                                    op=mybir.AluOpType.subtract)
            c_fake = sums.tile([128, 2], F32)
            nc.sync.dma_start(c_fake[:], fakecorr[:])
            nc.vector.tensor_tensor(out=st[:], in0=st[:], in1=c_fake[:],
                                    op=mybir.AluOpType.subtract)
            cin = dram.tile([128, 2], F32, tag="cin")
            cout = dram.tile([128, 2], F32, tag="cout")
            nc.sync.dma_start(cin[:], st[:])
            nc.gpsimd.collective_compute(
                "AllReduce", mybir.AluOpType.add,
                replica_groups=[list(range(NCORES))],
                ins=[cin.opt()], outs=[cout.opt()])
            # fold the two strip halves after the collective (off the
            # pre-mesh critical path)
            glob = sums.tile([64, 2], F32)
            nc.sync.dma_start(glob[:], cout[0:64, :])
            globB = sums.tile([64, 2], F32)
            nc.scalar.dma_start(globB[:], cout[64:128, :])
            nc.vector.tensor_tensor(out=glob[:], in0=glob[:], in1=globB[:],
                                    op=mybir.AluOpType.add)

            mean = sums.tile([64, 1], F32)
            nc.vector.tensor_scalar_mul(out=mean[:], in0=glob[:, 0:1],
                                        scalar1=1.0 / N_TOT)
            ex2 = sums.tile([64, 1], F32)
            nc.vector.tensor_scalar_mul(out=ex2[:], in0=glob[:, 1:2],
                                        scalar1=1.0 / N_TOT)
            var = sums.tile([64, 1], F32)
            nc.vector.tensor_tensor(out=var[:], in0=mean[:], in1=mean[:],
                                    op=mybir.AluOpType.mult)
            nc.vector.tensor_tensor(out=var[:], in0=ex2[:], in1=var[:],
                                    op=mybir.AluOpType.subtract)
            nc.vector.tensor_scalar_add(out=var[:], in0=var[:], scalar1=BN_EPS)
            sd = sums.tile([64, 1], F32)
            nc.scalar.activation(out=sd[:], in_=var[:],
                                 func=mybir.ActivationFunctionType.Sqrt)
            nc.vector.reciprocal(out=sd[:], in_=sd[:])
            c_g2 = sums.tile([128, 1], F32)
            nc.sync.dma_start(c_g2[:], gamma2[:])
            c_b2 = sums.tile([128, 1], F32)
            nc.sync.dma_start(c_b2[:], beta2[:])
            ab = sums.tile([64, 2], F32)
            nc.vector.tensor_tensor(out=ab[:, 0:1], in0=c_g2[0:64, :], in1=sd[:],
                                    op=mybir.AluOpType.mult)
            nc.vector.tensor_tensor(out=ab[:, 1:2], in0=mean[:], in1=ab[:, 0:1],
                                    op=mybir.AluOpType.mult)
            nc.vector.tensor_tensor(out=ab[:, 1:2], in0=c_b2[0:64, :],
                                    in1=ab[:, 1:2], op=mybir.AluOpType.subtract)
            ab2 = sums.tile([128, 2], F32)
            nc.sync.dma_start(ab2[0:64, :], ab[:])
            nc.gpsimd.dma_start(ab2[64:128, :], ab[:])

            # pass 4: out = relu(A*h + B), split DVE (f16 4x) / Act
            groups = []
            g = 0
            while g < nchunks:
                gr = 4 if g < (2 * nchunks) // 3 else 2
                groups.append((g, min(gr, nchunks - g)))
                g += gr
            with tc.tile_pool(name="otp", bufs=6) as otp:
                for gi, (g, GR) in enumerate(groups):
                    grp = schedule[g:g + GR]
                    goff = grp[0][0]
                    span = sum(c[1] for c in grp)
                    ot = otp.tile([128, 4 * LMAX], F16, tag="ot")
                    use_act = gi % 5 in (2, 4)
                    for idx, (off, L, v, nw) in enumerate(grp):
                        ht = state[g + idx]
                        co = off - goff
                        if use_act:
                            nc.scalar.activation(
                                out=ot[:, co:co + L], in_=ht[:, :L],
                                func=mybir.ActivationFunctionType.Relu,
                                scale=ab2[:, 0:1], bias=ab2[:, 1:2])
                        else:
                            nc.vector.tensor_scalar(
                                out=ot[:, co:co + L], in0=ht[:, :L],
                                scalar1=ab2[:, 0:1], scalar2=ab2[:, 1:2],
                                op0=mybir.AluOpType.mult,
                                op1=mybir.AluOpType.add)
                            nc.vector.tensor_scalar_max(
                                out=ot[:, co:co + L], in0=ot[:, co:co + L],
                                scalar1=0.0)
                    nc.sync.dma_start(hout[:, goff:goff + span], ot[:, :span])

    nc.compile()
    return nc


# ------------------------------------------------------------------- kernel
_CACHE = {}


def _prepare(pos, x, cluster, Wv, bv, Wk, bk, Wq, bq, gamma, beta):
    x = np.ascontiguousarray(np.asarray(x, np.float32))
    cluster = np.asarray(cluster).astype(np.int64)

    prep = _host_prep(cluster)
    W, schedule, nwtot = prep["W"], prep["schedule"], prep["nwtot"]

    key = (W, tuple(schedule))
    if key not in _CACHE:
        _CACHE[key] = _build_program(W, schedule, nwtot)
    nc = _CACHE[key]

    e2big = np.zeros((128, 128), np.float32)
    e2big[0:64, 0:64] = 1.0
    e2big[64:128, 64:128] = 1.0
    Wkf = np.asarray(Wk, np.float32)
    bqf = np.asarray(bq, np.float32)
    bvf = np.asarray(bv, np.float32)
    shared = dict(
        wqt=np.ascontiguousarray(np.vstack([np.asarray(Wq, np.float32).T] * 2)).astype(BF),
        wk2=np.ascontiguousarray(np.vstack([Wkf] * 2)).astype(BF),
        wvt=np.ascontiguousarray(np.vstack([np.asarray(Wv, np.float32).T] * 2)).astype(BF),
        e2big=e2big.astype(BF),
        bq2=np.tile(bqf, 2).reshape(128, 1).copy(),
        bv2=np.tile(bvf, 2).reshape(128, 1).copy(),
        gamma2=np.tile(np.asarray(gamma, np.float32), 2).reshape(128, 1).copy(),
        beta2=np.tile(np.asarray(beta, np.float32), 2).reshape(128, 1).copy(),
    )

    xbf = x.astype(BF)
    in_maps = []
    lays = []
    for d in range(NCORES):
        xin = np.zeros((128, W), BF)
        npw = np.zeros((128, nwtot), FH)
        fc = np.zeros((128, 2), np.float32)
        lay = []
        for h in range(2):
            s = 2 * d + h
            slots, pts, npads, nfake = _strip_layout(prep, s)
            xin[64 * h: 64 * h + 64, slots] = xbf[pts].T
            npw[64 * h: 64 * h + 64, :] = npads.astype(FH)[None, :]
            bvh = np.tile(bvf, 2).reshape(128)[64 * h: 64 * h + 64]
            fc[64 * h: 64 * h + 64, 0] = nfake * bvh
            fc[64 * h: 64 * h + 64, 1] = nfake * bvh * bvh
            lay.append((slots, pts))
        m = dict(shared)
        m["xin"] = xin
        m["npadw"] = npw
        m["fakecorr"] = fc
        in_maps.append(m)
        lays.append(lay)

    return nc, in_maps, lays


def _finish(results, lays):
    out = np.empty((N_TOT, D), np.float32)
    for d in range(NCORES):
        h = np.asarray(results[d]["hout"], np.float32)
        for si in range(2):
            slots, pts = lays[d][si]
            out[pts] = h[si * 64:(si + 1) * 64, slots].T
    return out


def kernel(**inputs):
    nc, in_maps, lays = _prepare(**inputs)
    res = run_bass_kernel_spmd(nc, in_maps, core_ids=list(range(NCORES)),
                               **getattr(kernel, "run_kwargs", {}))
    kernel.last_results = res
    return _finish(res.results, lays)
